# revision 4
# baseline (speedup 1.0000x reference)
"""GATv2 layer (100k nodes, 800k edges, 8 heads x 16) on 8 Trainium2 cores, v2.

Key differences vs v1 (3.49 ms):
- Edge gathers use InstDMAGatherAnt (one SWDGE instruction per few thousand
  rows, ~1us Pool time) instead of per-128-row indirect DMAs (~1us EACH).
  int16 gather indices force <=32k-row tables, so the xl table is split into
  4 quarters and each group's edges are quarter-sorted; runs use shared
  (max-over-cores) lengths so the subtile->group map is SPMD-static, and
  sections pad to 128 only at (chunk, quarter) granularity (boundary subtiles
  span two groups and get one extra segment matmul).
- Tables are fp16 (halves gather bytes); the whole edge pipeline runs fp16 on
  DVE at 2x throughput (rel err ~2e-4 vs 2e-2 budget, validated in numpy).
- Self-loop edges never enter the gather stream: their contribution is
  computed from contiguous per-group table rows and added to the group
  accumulator during the epilogue.
- Attention weights ew are applied via broadcast APs (no b4 matmul), and the
  ELU's -1 is dropped (LayerNorm is invariant to constant shifts).
"""

import math

import numpy as np

P = 128
H, D = 8, 16
IN = 128
OUT = 128
NEG_SLOPE = 0.2
LN_EPS = 1e-5
DEN_EPS = 1e-16

N_CORES = 8
N_SPLIT = 4
G_CHUNK = 4
T_SUB = 8
# Per-engine SWDGE descriptor ring caps a single gather: num/16+1 descs must
# fit (~96 with the default 16KB scratch). 1024 idxs (65 descs) is proven.
MAX_GATHER = 1024


def _wrap16(vals):
    """[n] int16 -> [16, n/16] (idx i at [i%16, i//16]), n % 16 == 0."""
    n = len(vals)
    assert n % 16 == 0
    return np.asarray(vals, np.int16).reshape(n // 16, 16).T


# ---------------------------------------------------------------------------
# CPU preprocessing: static SPMD schedule + per-core sideband arrays
# ---------------------------------------------------------------------------

def _preprocess(edge_index, n_nodes):
    src = np.asarray(edge_index[0], np.int64)
    dst = np.asarray(edge_index[1], np.int64)
    order = np.argsort(dst, kind="stable")
    src, dst = src[order], dst[order]

    assert n_nodes % N_CORES == 0
    per = n_nodes // N_CORES
    n_groups = math.ceil(per / P)
    own_pad = n_groups * P
    q_rows = math.ceil(n_nodes / N_SPLIT)
    assert q_rows <= 32767

    # per-(core, group, quarter) runs: (quarter-local src, group-local dst)
    runs = {}
    for c in range(N_CORES):
        base = c * per
        gb = np.minimum(base + np.arange(n_groups + 1) * P, base + per)
        b = np.searchsorted(dst, gb)
        for g in range(n_groups):
            s = src[b[g] : b[g + 1]]
            d = dst[b[g] : b[g + 1]]
            q = s // q_rows
            o = np.argsort(q, kind="stable")
            s, d, q = s[o], d[o], q[o]
            qb = np.searchsorted(q, np.arange(N_SPLIT + 1))
            for qq in range(N_SPLIT):
                runs[(c, g, qq)] = (
                    (s[qb[qq] : qb[qq + 1]] - qq * q_rows).astype(np.int16),
                    (d[qb[qq] : qb[qq + 1]] - (base + g * P)).astype(np.int16),
                )

    # shared run lengths (max over cores)
    L = np.zeros((n_groups, N_SPLIT), dtype=np.int64)
    for g in range(n_groups):
        for qq in range(N_SPLIT):
            L[g, qq] = max(len(runs[(c, g, qq)][0]) for c in range(N_CORES))

    n_chunks = math.ceil(n_groups / G_CHUNK)
    chunks = []
    sidebands = [[] for _ in range(N_CORES)]
    for ch in range(n_chunks):
        gs = list(range(ch * G_CHUNK, min((ch + 1) * G_CHUNK, n_groups)))
        nw = len(gs)
        # quarter sections: shared slot spans per (g, q) run
        qsub = []
        spans = {}  # (g, qq) -> (section-local start slot, length)
        for qq in range(N_SPLIT):
            off = 0
            for g in gs:
                spans[(g, qq)] = (off, int(L[g, qq]))
                off += int(L[g, qq])
            qsub.append((off + P - 1) // P)
        n_edge = sum(qsub)
        n_sub = ((n_edge + T_SUB - 1) // T_SUB) * T_SUB
        pad_sub = n_sub - n_edge  # trailing all-pad subtiles (no matmuls)

        # segment matmuls: per subtile, groups overlapping it
        sec0 = np.cumsum([0] + qsub[:-1])  # subtile index of section starts
        mm = []  # (j, slot, grel, batched, start, stop) in execution order
        g_first = {}
        g_last = {}
        for qq in range(N_SPLIT):
            for g in gs:
                o, ln = spans[(g, qq)]
                if ln == 0:
                    continue
                j0 = int(sec0[qq]) + o // P
                j1 = int(sec0[qq]) + (o + ln - 1) // P
                for j in range(j0, j1 + 1):
                    mm.append([j, gs.index(g), g - gs[0]])
        mm.sort(key=lambda t: t[0])
        cnt = {}
        for t in mm:
            cnt[t[0]] = cnt.get(t[0], 0) + 1
        for t in mm:
            t.append(cnt[t[0]] == 1)  # batched s4 usable iff sole group
        for i, t in enumerate(mm):
            g = t[1]
            if g not in g_first:
                g_first[g] = i
            g_last[g] = i
        mm = [
            (j, slot, grel, bat, i == g_first[slot], i == g_last[slot])
            for i, (j, slot, grel, bat) in enumerate(mm)
        ]

        # xr gather splits
        xr_splits = []
        i0 = 0
        while i0 < n_sub:
            n = min(n_sub - i0, MAX_GATHER // P)
            xr_splits.append((i0, n))
            i0 += n

        chunks.append(
            dict(gs=gs, nw=nw, qsub=qsub, n_edge=n_edge, n_sub=n_sub,
                 mm=mm, xr_splits=xr_splits)
        )

        # ---- per-core sideband: xl idx (4 sections) | xr idx | one-hot
        #      selection planes (one [P,128] fp16 plane per segment matmul),
        #      all int16 columns on 128 partitions
        for c in range(N_CORES):
            xl_secs = []
            xr_flat = np.zeros(n_sub * P, np.int16)
            dl_rel = np.full(n_sub * P, -1, np.int64)
            gsl = np.full(n_sub * P, -1, np.int64)  # group slot per edge slot
            for qq in range(N_SPLIT):
                sec = np.zeros(qsub[qq] * P, np.int16)
                for g in gs:
                    o, ln = spans[(g, qq)]
                    if ln == 0:
                        continue
                    s_l, d_l = runs[(c, g, qq)]
                    n_e = len(s_l)
                    sec[o : o + n_e] = s_l
                    glob = int(sec0[qq]) * P + o
                    # xr table row is core-local: g*128 + d_l
                    xr_flat[glob : glob + n_e] = (g * P + d_l).astype(np.int16)
                    dl_rel[glob : glob + n_e] = d_l
                    gsl[glob : glob + n_e] = gs.index(g)
                xl_secs.append(np.tile(_wrap16(sec), (8, 1)))
            parts = xl_secs + [
                np.tile(_wrap16(xr_flat), (8, 1)),
                np.where(dl_rel < 0, -1.0, dl_rel).astype(np.float16
                    ).reshape(n_sub, P).T.view(np.int16),
                np.where(gsl < 0, -1.0, gsl * P + dl_rel).astype(np.float16
                    ).reshape(n_sub, P).T.view(np.int16),
            ]
            sidebands[c].append(np.concatenate(parts, axis=1))

    sideband = [np.concatenate(s, axis=1) for s in sidebands]
    return dict(per=per, n_groups=n_groups, own_pad=own_pad, q_rows=q_rows,
                chunks=chunks, sideband=sideband)


# ---------------------------------------------------------------------------
# Bass program
# ---------------------------------------------------------------------------

def _build_program(n_nodes, pp, use_bias, use_gamma, use_beta):
    from contextlib import ExitStack

    from concourse import bass, mybir
    from concourse import tile as tile_mod
    from concourse.bacc import Bacc

    f32 = mybir.dt.float32
    f16 = mybir.dt.float16
    i16 = mybir.dt.int16
    Alu = mybir.AluOpType
    Act = mybir.ActivationFunctionType
    TileContext = tile_mod.TileContext

    per = pp["per"]
    n_groups = pp["n_groups"]
    own_pad = pp["own_pad"]
    q_rows = pp["q_rows"]
    chunks = pp["chunks"]
    nsub_max = max(ch["n_sub"] for ch in chunks)
    sbw_max = max(ch["n_edge"] * 8 + ch["n_sub"] * 10 for ch in chunks)
    sb_w_total = sum(ch["n_edge"] * 8 + ch["n_sub"] * 10 for ch in chunks)

    nc = Bacc(num_swdge_queues=4)

    x_full = nc.declare_dram_parameter("x_full", [n_nodes, IN], f32, isOutput=False)
    x_own = nc.declare_dram_parameter("x_own", [own_pad, IN], f32, isOutput=False)
    w_l = nc.declare_dram_parameter("w_l", [IN, OUT], f32, isOutput=False)
    w_r = nc.declare_dram_parameter("w_r", [IN, OUT], f32, isOutput=False)
    sb_d = nc.declare_dram_parameter("sb", [P, sb_w_total], i16, isOutput=False)
    att_d = nc.declare_dram_parameter("att_b", [P, OUT], f16, isOutput=False)
    iota_d = nc.declare_dram_parameter("iota_b", [P, P], f16, isOutput=False)
    iota8_d = nc.declare_dram_parameter("iota8_b", [P, T_SUB * P], f16,
                                        isOutput=False)
    ident_d = nc.declare_dram_parameter("ident", [P, P], f32, isOutput=False)
    aff_d = nc.declare_dram_parameter("aff", [P, 3 * OUT], f32, isOutput=False)
    eps_d = nc.declare_dram_parameter("eps_b", [P, 1], f32, isOutput=False)
    out_own = nc.declare_dram_parameter("out_own", [own_pad, OUT], f32,
                                        isOutput=True)

    xl_tab = nc.dram_tensor("xl_tab", [n_nodes, OUT], f16)
    xlo_tab = nc.dram_tensor("xlo_tab", [own_pad, OUT], f16)
    xr_tab = nc.dram_tensor("xr_tab", [own_pad, OUT], f16)

    with TileContext(nc) as tc, ExitStack() as ctx:
        const = ctx.enter_context(tc.tile_pool(name="const", bufs=1))
        wl_s = const.tile([IN, OUT], f32)
        wr_s = const.tile([IN, OUT], f32)
        att_s = const.tile([P, OUT], f16)
        iota_s = const.tile([P, P], f16)
        iota8_s = const.tile([P, T_SUB * P], f16)
        ident_s = const.tile([P, P], f32)
        nc.sync.dma_start(out=wl_s[:], in_=w_l[:])
        nc.sync.dma_start(out=wr_s[:], in_=w_r[:])
        nc.sync.dma_start(out=att_s[:], in_=att_d[:])
        nc.sync.dma_start(out=iota_s[:], in_=iota_d[:])
        nc.sync.dma_start(out=iota8_s[:], in_=iota8_d[:])
        nc.sync.dma_start(out=ident_s[:], in_=ident_d[:])
        aff_s = const.tile([P, 3 * OUT], f32)
        nc.sync.dma_start(out=aff_s[:], in_=aff_d[:])
        eps_s = const.tile([P, 1], f32)
        nc.sync.dma_start(out=eps_s[:], in_=eps_d[:])

        # PE warmup: observe DMA-loaded constants once on the PE clock
        with tc.tile_pool(name="warm", bufs=1, space="PSUM") as warm:
            warm_p = warm.tile([P, P], f32)
            nc.tensor.transpose(out=warm_p[:], in_=ident_s[:],
                                identity=ident_s[:])
            nc.tensor.matmul(out=warm_p[:, :OUT], lhsT=ident_s[:],
                             rhs=wl_s[:], start=True, stop=True)
            nc.tensor.matmul(out=warm_p[:, :OUT], lhsT=ident_s[:],
                             rhs=wr_s[:], start=True, stop=True)

        # ------------------------------------------------------------------
        # Phase 1: projection tables (fp16)
        # ------------------------------------------------------------------
        def project(src_ap, n_rows, outs):
            # outs: list of (w_tile, dst_dram)
            with tc.tile_pool(name="p1", bufs=3) as p1, \
                 tc.tile_pool(name="p1ps", bufs=2, space="PSUM") as p1ps:
                n_blk = (n_rows + 511) // 512
                for b in range(n_blk):
                    r0 = b * 512
                    rows = min(512, n_rows - r0)
                    n_j = (rows + P - 1) // P
                    xin = p1.tile([P, 4, IN], f32, tag="xin")
                    if rows == 512:
                        nc.sync.dma_start(
                            out=xin[:],
                            in_=src_ap[r0 : r0 + 512, :].rearrange(
                                "(j p) f -> p j f", p=P),
                        )
                    else:
                        for j in range(n_j):
                            jr = min(P, rows - j * P)
                            nc.sync.dma_start(
                                out=xin[:jr, j, :],
                                in_=src_ap[r0 + j * P : r0 + j * P + jr, :])
                    stgs = [p1.tile([P, 4, OUT], f16, tag=f"stg{k}",
                                    name=f"stg{k}_{b}")
                            for k in range(len(outs))]
                    o_ps = [p1ps.tile([P, 4, OUT], f32, tag=f"op{k}",
                                      name=f"op{k}_{b}")
                            for k in range(len(outs))]
                    for j in range(n_j):
                        jr = min(P, rows - j * P)
                        xt_p = p1ps.tile([P, P], f32, tag="xt")
                        nc.tensor.transpose(
                            out=xt_p[:, :jr], in_=xin[:jr, j, :],
                            identity=ident_s[:jr, :jr])
                        xt_s = p1.tile([P, P], f32, tag="xts")
                        nc.scalar.copy(out=xt_s[:, :jr], in_=xt_p[:, :jr])
                        for k, (w_tile, _) in enumerate(outs):
                            # one accumulation group per PSUM bank: start
                            # zeroes the whole 2KB zero-region, so only j==0
                            # may set it
                            nc.tensor.matmul(
                                out=o_ps[k][:jr, j, :], lhsT=xt_s[:, :jr],
                                rhs=w_tile[:], start=(j == 0),
                                stop=(j == n_j - 1))
                    for k in range(len(outs)):
                        nc.scalar.copy(out=stgs[k][:, :n_j, :],
                                       in_=o_ps[k][:, :n_j, :])
                    for k, (_, dst) in enumerate(outs):
                        if rows == 512:
                            nc.sync.dma_start(
                                out=dst[r0 : r0 + 512, :].rearrange(
                                    "(j p) f -> p j f", p=P),
                                in_=stgs[k][:])
                        else:
                            for j in range(n_j):
                                jr = min(P, rows - j * P)
                                nc.sync.dma_start(
                                    out=dst[r0 + j * P : r0 + j * P + jr, :],
                                    in_=stgs[k][:jr, j, :])

        project(x_full[:], n_nodes, [(wl_s, xl_tab[:])])
        project(x_own[:], own_pad, [(wl_s, xlo_tab[:]), (wr_s, xr_tab[:])])

        # ------------------------------------------------------------------
        # Phase 2: chunked edge processing
        # ------------------------------------------------------------------
        with tc.tile_pool(name="p2", bufs=3) as p2, \
             tc.tile_pool(name="stg2", bufs=2) as stg2, \
             tc.tile_pool(name="gps", bufs=2, space="PSUM") as gps:

            att_b8 = att_s[:][:, None, :].to_broadcast((P, T_SUB, OUT))
            iota_b8 = iota8_s[:].rearrange("p (j d) -> p j d", j=T_SUB)

            state = {"gq": 0}
            sb_off = 0
            for ci, ch in enumerate(chunks):
                gs, nw = ch["gs"], ch["nw"]
                g0 = gs[0]
                n_sub = ch["n_sub"]
                qsub = ch["qsub"]
                w_ch = ch["n_edge"] * 8 + n_sub * 10
                sb_t = stg2.tile([P, sbw_max], i16, tag="sb")
                nc.sync.dma_start(out=sb_t[:, :w_ch],
                                  in_=sb_d[:, sb_off : sb_off + w_ch])
                sb_off += w_ch

                xl_st = stg2.tile([P, nsub_max, OUT], f16, tag="xl")
                xr_st = stg2.tile([P, nsub_max, OUT], f16, tag="xr")
                cap = MAX_GATHER // P
                # gathers rotate across the 4 SWDGE queues so queue-ring
                # reclaim (waiting on the previous gather's DMA) overlaps
                # with the other queues' descriptor generation
                # xl gathers (per quarter section, capped)
                off_w = 0
                si = 0
                for qq in range(N_SPLIT):
                    qs = qsub[qq]
                    src = xl_tab[qq * q_rows : min((qq + 1) * q_rows,
                                                   n_nodes), :]
                    for o in range(0, qs, cap):
                        n = min(cap, qs - o)
                        nc.gpsimd.dma_gather(
                            xl_st[:, si + o : si + o + n, :], src,
                            sb_t[:, off_w + o * 8 : off_w + (o + n) * 8],
                            n * P, n * P, OUT,
                            queue_num=state["gq"] % 4)
                        state["gq"] += 1
                    off_w += qs * 8
                    si += qs
                # xr gathers (capped)
                for o in range(0, n_sub, cap):
                    n = min(cap, n_sub - o)
                    nc.gpsimd.dma_gather(
                        xr_st[:, o : o + n, :], xr_tab[:],
                        sb_t[:, off_w + o * 8 : off_w + (o + n) * 8],
                        n * P, n * P, OUT,
                        queue_num=state["gq"] % 4)
                    state["gq"] += 1
                off_w += n_sub * 8
                dl_rel = sb_t[:, off_w : off_w + n_sub].bitcast(f16)
                off_w += n_sub
                dl_chunk = sb_t[:, off_w : off_w + n_sub].bitcast(f16)

                # self blocks (own-group rows, contiguous)
                xl_sf = p2.tile([P, G_CHUNK, OUT], f16, tag="xlsf")
                xr_sf = p2.tile([P, G_CHUNK, OUT], f16, tag="xrsf")
                nc.sync.dma_start(
                    out=xl_sf[:, :nw, :],
                    in_=xlo_tab[g0 * P : (g0 + nw) * P, :].rearrange(
                        "(j p) f -> p j f", p=P))
                nc.sync.dma_start(
                    out=xr_sf[:, :nw, :],
                    in_=xr_tab[g0 * P : (g0 + nw) * P, :].rearrange(
                        "(j p) f -> p j f", p=P))
                m_s = p2.tile([P, G_CHUNK, OUT], f16, tag="ms")
                nc.vector.tensor_tensor(out=m_s[:, :nw, :],
                                        in0=xl_sf[:, :nw, :],
                                        in1=xr_sf[:, :nw, :], op=Alu.add)
                nc.vector.scalar_tensor_tensor(
                    out=m_s[:, :nw, :], in0=m_s[:, :nw, :], scalar=NEG_SLOPE,
                    in1=m_s[:, :nw, :], op0=Alu.mult, op1=Alu.max)
                u_s = p2.tile([P, G_CHUNK, OUT], f16, tag="us")
                nc.vector.tensor_tensor(
                    out=u_s[:, :nw, :], in0=m_s[:, :nw, :],
                    in1=att_s[:][:, None, :].to_broadcast((P, nw, OUT)),
                    op=Alu.mult)
                e_s = p2.tile([P, G_CHUNK, H], f32, tag="es")
                nc.vector.tensor_reduce(
                    out=e_s[:, :nw, :].rearrange("p j h -> p (j h)"),
                    in_=u_s[:, :nw, :].rearrange("p j (h d) -> p (j h) d", h=H),
                    axis=mybir.AxisListType.X, op=Alu.add)
                ex_s = p2.tile([P, G_CHUNK, H], f16, tag="exs")
                nc.scalar.activation(out=ex_s[:, :nw, :], in_=e_s[:, :nw, :],
                                     func=Act.Exp)
                swt = p2.tile([P, G_CHUNK, OUT], f32, tag="swt")
                nc.vector.tensor_tensor(
                    out=swt[:, :nw, :].rearrange(
                        "p j (h d) -> p j h d", h=H),
                    in0=xl_sf[:, :nw, :].rearrange(
                        "p j (h d) -> p j h d", h=H),
                    in1=ex_s[:, :nw, :, None].to_broadcast((P, nw, H, D)),
                    op=Alu.mult)

                g_all = gps.tile([P, G_CHUNK, OUT], f32, tag="gall",
                                 name=f"gall{ci}")
                g_psum = [g_all[:, k, :] for k in range(nw)]
                g_den = gps.tile([P, G_CHUNK, H], f32, tag="gden",
                                 name=f"gden{ci}")

                # edge big-tiles
                mm = ch["mm"]
                mi = 0
                for bt in range(n_sub // T_SUB):
                    j0 = bt * T_SUB
                    sl = slice(j0, j0 + T_SUB)
                    m = p2.tile([P, T_SUB, OUT], f16, tag="m")
                    nc.vector.tensor_tensor(out=m[:], in0=xl_st[:, sl, :],
                                            in1=xr_st[:, sl, :], op=Alu.add)
                    t_l = p2.tile([P, T_SUB, OUT], f16, tag="tl")
                    nc.vector.scalar_tensor_tensor(
                        out=t_l[:], in0=m[:], scalar=NEG_SLOPE, in1=m[:],
                        op0=Alu.mult, op1=Alu.max)
                    u = p2.tile([P, T_SUB, OUT], f16, tag="u")
                    nc.vector.tensor_tensor(out=u[:], in0=t_l[:], in1=att_b8,
                                            op=Alu.mult)
                    e = p2.tile([P, T_SUB, H], f32, tag="e")
                    nc.vector.tensor_reduce(
                        out=e[:].rearrange("p j h -> p (j h)"),
                        in_=u[:].rearrange("p j (h d) -> p (j h) d", h=H),
                        axis=mybir.AxisListType.X, op=Alu.add)
                    ex = p2.tile([P, T_SUB, H], f16, tag="ex")
                    nc.scalar.activation(out=ex[:], in_=e[:], func=Act.Exp)
                    w_t = p2.tile([P, T_SUB, OUT], f16, tag="wt")
                    nc.vector.tensor_tensor(
                        out=w_t[:].rearrange("p j (h d) -> p j h d", h=H),
                        in0=xl_st[:, sl, :].rearrange(
                            "p j (h d) -> p j h d", h=H),
                        in1=ex[:, :, :, None].to_broadcast((P, T_SUB, H, D)),
                        op=Alu.mult)
                    s4b = p2.tile([P, T_SUB, P], f16, tag="s4b")
                    nc.vector.tensor_tensor(
                        out=s4b[:], in0=iota_b8,
                        in1=dl_rel[:, sl][:, :, None].to_broadcast(
                            (P, T_SUB, P)),
                        op=Alu.is_equal)
                    while mi < len(mm) and mm[mi][0] < j0 + T_SUB:
                        j, slot, grel, bat, st, sp = mm[mi]
                        if bat:
                            lhsT = s4b[:, j - j0, :]
                        else:
                            s4x = p2.tile([P, P], f16, tag="s4x")
                            nc.vector.scalar_tensor_tensor(
                                out=s4x[:],
                                in0=dl_chunk[:, j][:, None].to_broadcast(
                                    (P, P)),
                                scalar=float(-grel * P),
                                in1=iota_s[:], op0=Alu.add, op1=Alu.is_equal)
                            lhsT = s4x[:]
                        # g_all/g_den hold all 4 groups in one PSUM bank
                        # each: a single accumulation group per bank (start
                        # zeroes the whole 2KB zero-region)
                        st = mi == 0
                        sp = mi == len(mm) - 1
                        nc.tensor.matmul(out=g_psum[slot], lhsT=lhsT,
                                         rhs=w_t[:, j - j0, :],
                                         start=st, stop=sp)
                        # denominator rides the same stationary lhsT
                        nc.tensor.matmul(out=g_den[:, slot, :], lhsT=lhsT,
                                         rhs=ex[:, j - j0, :],
                                         start=st, stop=sp)
                        mi += 1
                assert mi == len(mm)

                # ---- epilogue for this chunk's groups
                stage = p2.tile([P, G_CHUNK, OUT], f32, tag="stage")
                nc.vector.tensor_tensor(out=stage[:, :nw, :],
                                        in0=g_all[:, :nw, :],
                                        in1=swt[:, :nw, :], op=Alu.add)
                rd = p2.tile([P, G_CHUNK, H], f32, tag="rd")
                nc.vector.tensor_tensor(out=rd[:, :nw, :],
                                        in0=g_den[:, :nw, :],
                                        in1=ex_s[:, :nw, :], op=Alu.add)
                nc.vector.tensor_scalar_add(rd[:, :nw, :], rd[:, :nw, :],
                                            DEN_EPS)
                nc.vector.reciprocal(rd[:, :nw, :], rd[:, :nw, :])
                o1 = p2.tile([P, G_CHUNK, OUT], f32, tag="o1")
                nc.vector.tensor_tensor(
                    out=o1[:, :nw, :].rearrange("p j (h d) -> p j h d", h=H),
                    in0=stage[:, :nw, :].rearrange("p j (h d) -> p j h d",
                                                   h=H),
                    in1=rd[:, :nw, :, None].to_broadcast((P, nw, H, D)),
                    op=Alu.mult)
                if use_bias:
                    nc.vector.tensor_tensor(
                        out=o1[:, :nw, :], in0=o1[:, :nw, :],
                        in1=aff_s[:][:, None, 2 * OUT : 3 * OUT].to_broadcast(
                            (P, nw, OUT)),
                        op=Alu.add)
                xres = p2.tile([P, G_CHUNK, OUT], f32, tag="xres")
                nc.sync.dma_start(
                    out=xres[:, :nw, :],
                    in_=x_own[g0 * P : (g0 + nw) * P, :].rearrange(
                        "(j p) f -> p j f", p=P))
                # ELU(o1) + x  (the ELU -1 is dropped: LN is shift-invariant)
                vmin = p2.tile([P, G_CHUNK, OUT], f32, tag="vmin")
                nc.vector.tensor_scalar_min(vmin[:, :nw, :], o1[:, :nw, :],
                                            0.0)
                ev = p2.tile([P, G_CHUNK, OUT], f32, tag="ev")
                nc.scalar.activation(out=ev[:, :nw, :], in_=vmin[:, :nw, :],
                                     func=Act.Exp)
                v = p2.tile([P, G_CHUNK, OUT], f32, tag="v")
                nc.vector.scalar_tensor_tensor(
                    out=v[:, :nw, :], in0=o1[:, :nw, :], scalar=0.0,
                    in1=xres[:, :nw, :], op0=Alu.max, op1=Alu.add)
                nc.vector.tensor_tensor(out=v[:, :nw, :], in0=v[:, :nw, :],
                                        in1=ev[:, :nw, :], op=Alu.add)
                mu = p2.tile([P, G_CHUNK], f32, tag="mu")
                nc.vector.tensor_reduce(out=mu[:, :nw], in_=v[:, :nw, :],
                                        axis=mybir.AxisListType.X, op=Alu.add)
                nc.scalar.mul(out=mu[:, :nw], in_=mu[:, :nw], mul=1.0 / OUT)
                cen = p2.tile([P, G_CHUNK, OUT], f32, tag="cen")
                nc.vector.tensor_tensor(
                    out=cen[:, :nw, :], in0=v[:, :nw, :],
                    in1=mu[:, :nw, None].to_broadcast((P, nw, OUT)),
                    op=Alu.subtract)
                sq = p2.tile([P, G_CHUNK, OUT], f32, tag="sq")
                nc.vector.tensor_tensor(out=sq[:, :nw, :], in0=cen[:, :nw, :],
                                        in1=cen[:, :nw, :], op=Alu.mult)
                var = p2.tile([P, G_CHUNK], f32, tag="var")
                nc.vector.tensor_reduce(out=var[:, :nw], in_=sq[:, :nw, :],
                                        axis=mybir.AxisListType.X, op=Alu.add)
                # std = sqrt(var/OUT + eps) in one ACT op
                nc.scalar.activation(out=var[:, :nw], in_=var[:, :nw],
                                     func=Act.Sqrt, scale=1.0 / OUT,
                                     bias=eps_s[:, 0:1])
                nc.vector.reciprocal(var[:, :nw], var[:, :nw])
                o2 = p2.tile([P, G_CHUNK, OUT], f32, tag="o2")
                nc.vector.tensor_tensor(
                    out=o2[:, :nw, :], in0=cen[:, :nw, :],
                    in1=var[:, :nw, None].to_broadcast((P, nw, OUT)),
                    op=Alu.mult)
                if use_gamma:
                    nc.vector.tensor_tensor(
                        out=o2[:, :nw, :], in0=o2[:, :nw, :],
                        in1=aff_s[:][:, None, 0:OUT].to_broadcast((P, nw, OUT)),
                        op=Alu.mult)
                if use_beta:
                    nc.vector.tensor_tensor(
                        out=o2[:, :nw, :], in0=o2[:, :nw, :],
                        in1=aff_s[:][:, None, OUT : 2 * OUT].to_broadcast(
                            (P, nw, OUT)),
                        op=Alu.add)
                nc.sync.dma_start(
                    out=out_own[g0 * P : (g0 + nw) * P, :].rearrange(
                        "(j p) f -> p j f", p=P),
                    in_=o2[:, :nw, :])

    nc.finalize()
    return nc


# ---------------------------------------------------------------------------
# Host entry point
# ---------------------------------------------------------------------------

TRACE = False
LAST = {}


def kernel(x, edge_index, W_l, b_l, W_r, b_r, att, bias, gamma, beta):
    from concourse.bass_utils import run_bass_kernel_spmd

    x = np.asarray(x, dtype=np.float32)
    n_nodes = x.shape[0]

    pp = _preprocess(np.asarray(edge_index), n_nodes)
    per, own_pad = pp["per"], pp["own_pad"]

    use_bias = bool(np.any(bias))
    use_gamma = bool(np.any(np.asarray(gamma) != 1.0))
    use_beta = bool(np.any(beta))
    # b_l/b_r fold into the tables via host-side? They are zeros in this
    # problem; fall back to adding on host if nonzero.
    assert not np.any(b_l) and not np.any(b_r), "nonzero proj bias unsupported"

    nc = _build_program(n_nodes, pp, use_bias, use_gamma, use_beta)

    att_b = np.tile(np.asarray(att, np.float16).reshape(1, OUT), (P, 1))
    iota_b = np.tile(np.arange(P, dtype=np.float16)[None, :], (P, 1))
    iota8_b = np.tile(np.arange(P, dtype=np.float16)[None, :], (P, T_SUB))
    ident = np.eye(P, dtype=np.float32)
    aff = np.zeros((P, 3 * OUT), dtype=np.float32)
    aff[:, 0:OUT] = np.asarray(gamma, np.float32)[None, :]
    aff[:, OUT : 2 * OUT] = np.asarray(beta, np.float32)[None, :]
    aff[:, 2 * OUT : 3 * OUT] = np.asarray(bias, np.float32)[None, :]

    in_maps = []
    for c in range(N_CORES):
        x_own = np.zeros((own_pad, IN), dtype=np.float32)
        x_own[:per] = x[c * per : (c + 1) * per]
        in_maps.append({
            "x_full": x,
            "x_own": x_own,
            "w_l": np.asarray(W_l, dtype=np.float32),
            "w_r": np.asarray(W_r, dtype=np.float32),
            "sb": pp["sideband"][c],
            "att_b": att_b,
            "iota_b": iota_b,
            "iota8_b": iota8_b,
            "ident": ident,
            "aff": aff,
            "eps_b": np.full((P, 1), LN_EPS, dtype=np.float32),
        })

    res = run_bass_kernel_spmd(nc, in_maps, list(range(N_CORES)), trace=TRACE)
    LAST["res"] = res
    outs = [res.results[c]["out_own"][:per] for c in range(N_CORES)]
    return np.concatenate(outs, axis=0).astype(np.float32)


# revision 7
# speedup vs baseline: 1.0378x; 1.0378x over previous
"""GATv2 layer (100k nodes, 800k edges, 8 heads x 16) on 8 Trainium2 cores, v2.

Key differences vs v1 (3.49 ms):
- Edge gathers use InstDMAGatherAnt (one SWDGE instruction per few thousand
  rows, ~1us Pool time) instead of per-128-row indirect DMAs (~1us EACH).
  int16 gather indices force <=32k-row tables, so the xl table is split into
  4 quarters and each group's edges are quarter-sorted; runs use shared
  (max-over-cores) lengths so the subtile->group map is SPMD-static, and
  sections pad to 128 only at (chunk, quarter) granularity (boundary subtiles
  span two groups and get one extra segment matmul).
- Tables are fp16 (halves gather bytes); the whole edge pipeline runs fp16 on
  DVE at 2x throughput (rel err ~2e-4 vs 2e-2 budget, validated in numpy).
- Self-loop edges never enter the gather stream: their contribution is
  computed from contiguous per-group table rows and added to the group
  accumulator during the epilogue.
- Attention weights ew are applied via broadcast APs (no b4 matmul), and the
  ELU's -1 is dropped (LayerNorm is invariant to constant shifts).
"""

import math

import numpy as np

P = 128
H, D = 8, 16
IN = 128
OUT = 128
NEG_SLOPE = 0.2
LN_EPS = 1e-5
DEN_EPS = 1e-16

N_CORES = 8
N_SPLIT = 4
G_CHUNK = 4
T_SUB = 8
# Per-engine SWDGE descriptor ring caps a single gather: num/16+1 descs must
# fit (~96 with the default 16KB scratch). 1024 idxs (65 descs) is proven.
MAX_GATHER = 1024


def _wrap16(vals):
    """[n] int16 -> [16, n/16] (idx i at [i%16, i//16]), n % 16 == 0."""
    n = len(vals)
    assert n % 16 == 0
    return np.asarray(vals, np.int16).reshape(n // 16, 16).T


# ---------------------------------------------------------------------------
# CPU preprocessing: static SPMD schedule + per-core sideband arrays
# ---------------------------------------------------------------------------

def _preprocess(edge_index, n_nodes):
    src = np.asarray(edge_index[0], np.int64)
    dst = np.asarray(edge_index[1], np.int64)
    order = np.argsort(dst, kind="stable")
    src, dst = src[order], dst[order]

    assert n_nodes % N_CORES == 0
    per = n_nodes // N_CORES
    n_groups = math.ceil(per / P)
    own_pad = n_groups * P
    q_rows = math.ceil(n_nodes / N_SPLIT)
    assert q_rows <= 32767

    # per-(core, group, quarter) runs: (quarter-local src, group-local dst)
    runs = {}
    for c in range(N_CORES):
        base = c * per
        gb = np.minimum(base + np.arange(n_groups + 1) * P, base + per)
        b = np.searchsorted(dst, gb)
        for g in range(n_groups):
            s = src[b[g] : b[g + 1]]
            d = dst[b[g] : b[g + 1]]
            q = s // q_rows
            o = np.argsort(q, kind="stable")
            s, d, q = s[o], d[o], q[o]
            qb = np.searchsorted(q, np.arange(N_SPLIT + 1))
            for qq in range(N_SPLIT):
                runs[(c, g, qq)] = (
                    (s[qb[qq] : qb[qq + 1]] - qq * q_rows).astype(np.int16),
                    (d[qb[qq] : qb[qq + 1]] - (base + g * P)).astype(np.int16),
                )

    # shared run lengths (max over cores)
    L = np.zeros((n_groups, N_SPLIT), dtype=np.int64)
    for g in range(n_groups):
        for qq in range(N_SPLIT):
            L[g, qq] = max(len(runs[(c, g, qq)][0]) for c in range(N_CORES))

    n_chunks = math.ceil(n_groups / G_CHUNK)
    chunks = []
    sidebands = [[] for _ in range(N_CORES)]
    for ch in range(n_chunks):
        gs = list(range(ch * G_CHUNK, min((ch + 1) * G_CHUNK, n_groups)))
        nw = len(gs)
        # quarter sections: shared slot spans per (g, q) run
        qsub = []
        spans = {}  # (g, qq) -> (section-local start slot, length)
        for qq in range(N_SPLIT):
            off = 0
            for g in gs:
                spans[(g, qq)] = (off, int(L[g, qq]))
                off += int(L[g, qq])
            qsub.append((off + P - 1) // P)
        n_edge = sum(qsub)
        n_sub = ((n_edge + T_SUB - 1) // T_SUB) * T_SUB
        pad_sub = n_sub - n_edge  # trailing all-pad subtiles (no matmuls)

        # segment matmuls: per subtile, groups overlapping it
        sec0 = np.cumsum([0] + qsub[:-1])  # subtile index of section starts
        mm = []  # (j, slot, grel, batched, start, stop) in execution order
        g_first = {}
        g_last = {}
        for qq in range(N_SPLIT):
            for g in gs:
                o, ln = spans[(g, qq)]
                if ln == 0:
                    continue
                j0 = int(sec0[qq]) + o // P
                j1 = int(sec0[qq]) + (o + ln - 1) // P
                for j in range(j0, j1 + 1):
                    mm.append([j, gs.index(g), g - gs[0]])
        mm.sort(key=lambda t: t[0])
        cnt = {}
        for t in mm:
            cnt[t[0]] = cnt.get(t[0], 0) + 1
        for t in mm:
            t.append(cnt[t[0]] == 1)  # batched s4 usable iff sole group
        for i, t in enumerate(mm):
            g = t[1]
            if g not in g_first:
                g_first[g] = i
            g_last[g] = i
        mm = [
            (j, slot, grel, bat, i == g_first[slot], i == g_last[slot])
            for i, (j, slot, grel, bat) in enumerate(mm)
        ]

        # xr gather splits
        xr_splits = []
        i0 = 0
        while i0 < n_sub:
            n = min(n_sub - i0, MAX_GATHER // P)
            xr_splits.append((i0, n))
            i0 += n

        chunks.append(
            dict(gs=gs, nw=nw, qsub=qsub, n_edge=n_edge, n_sub=n_sub,
                 mm=mm, xr_splits=xr_splits)
        )

        # ---- per-core sideband: xl idx (4 sections) | xr idx | one-hot
        #      selection planes (one [P,128] fp16 plane per segment matmul),
        #      all int16 columns on 128 partitions
        for c in range(N_CORES):
            xl_secs = []
            xr_flat = np.zeros(n_sub * P, np.int16)
            dl_rel = np.full(n_sub * P, -1, np.int64)
            gsl = np.full(n_sub * P, -1, np.int64)  # group slot per edge slot
            for qq in range(N_SPLIT):
                sec = np.zeros(qsub[qq] * P, np.int16)
                for g in gs:
                    o, ln = spans[(g, qq)]
                    if ln == 0:
                        continue
                    s_l, d_l = runs[(c, g, qq)]
                    n_e = len(s_l)
                    sec[o : o + n_e] = s_l
                    glob = int(sec0[qq]) * P + o
                    # xr table row is core-local: g*128 + d_l
                    xr_flat[glob : glob + n_e] = (g * P + d_l).astype(np.int16)
                    dl_rel[glob : glob + n_e] = d_l
                    gsl[glob : glob + n_e] = gs.index(g)
                xl_secs.append(np.tile(_wrap16(sec), (8, 1)))
            parts = xl_secs + [
                np.tile(_wrap16(xr_flat), (8, 1)),
                np.where(dl_rel < 0, -1.0, dl_rel).astype(np.float16
                    ).reshape(n_sub, P).T.view(np.int16),
                np.where(gsl < 0, -1.0, gsl * P + dl_rel).astype(np.float16
                    ).reshape(n_sub, P).T.view(np.int16),
            ]
            sidebands[c].append(np.concatenate(parts, axis=1))

    sideband = [np.concatenate(s, axis=1) for s in sidebands]
    return dict(per=per, n_groups=n_groups, own_pad=own_pad, q_rows=q_rows,
                chunks=chunks, sideband=sideband)


# ---------------------------------------------------------------------------
# Bass program
# ---------------------------------------------------------------------------

def _build_program(n_nodes, pp, use_bias, use_gamma, use_beta):
    from contextlib import ExitStack

    from concourse import bass, mybir
    from concourse import tile as tile_mod
    from concourse.bacc import Bacc

    f32 = mybir.dt.float32
    f16 = mybir.dt.float16
    i16 = mybir.dt.int16
    Alu = mybir.AluOpType
    Act = mybir.ActivationFunctionType
    TileContext = tile_mod.TileContext

    per = pp["per"]
    n_groups = pp["n_groups"]
    own_pad = pp["own_pad"]
    q_rows = pp["q_rows"]
    chunks = pp["chunks"]
    nsub_max = max(ch["n_sub"] for ch in chunks)
    sbw_max = max(ch["n_edge"] * 8 + ch["n_sub"] * 10 for ch in chunks)
    sb_w_total = sum(ch["n_edge"] * 8 + ch["n_sub"] * 10 for ch in chunks)

    nc = Bacc(num_swdge_queues=4)

    x_full = nc.declare_dram_parameter("x_full", [n_nodes, IN], f32, isOutput=False)
    x_own = nc.declare_dram_parameter("x_own", [own_pad, IN], f32, isOutput=False)
    w_l = nc.declare_dram_parameter("w_l", [IN, OUT], f32, isOutput=False)
    w_r = nc.declare_dram_parameter("w_r", [IN, OUT], f32, isOutput=False)
    sb_d = nc.declare_dram_parameter("sb", [P, sb_w_total], i16, isOutput=False)
    att_d = nc.declare_dram_parameter("att_b", [P, OUT], f16, isOutput=False)
    iota_d = nc.declare_dram_parameter("iota_b", [P, P], f16, isOutput=False)
    iota8_d = nc.declare_dram_parameter("iota8_b", [P, T_SUB * P], f16,
                                        isOutput=False)
    ident_d = nc.declare_dram_parameter("ident", [P, P], f32, isOutput=False)
    aff_d = nc.declare_dram_parameter("aff", [P, 3 * OUT], f32, isOutput=False)
    eps_d = nc.declare_dram_parameter("eps_b", [P, 1], f32, isOutput=False)
    out_own = nc.declare_dram_parameter("out_own", [own_pad, OUT], f32,
                                        isOutput=True)

    xl_tab = nc.dram_tensor("xl_tab", [n_nodes, OUT], f16)
    xlo_tab = nc.dram_tensor("xlo_tab", [own_pad, OUT], f16)
    xr_tab = nc.dram_tensor("xr_tab", [own_pad, OUT], f16)

    with TileContext(nc) as tc, ExitStack() as ctx:
        const = ctx.enter_context(tc.tile_pool(name="const", bufs=1))
        wl_s = const.tile([IN, OUT], f32)
        wr_s = const.tile([IN, OUT], f32)
        att_s = const.tile([P, OUT], f16)
        iota_s = const.tile([P, P], f16)
        iota8_s = const.tile([P, T_SUB * P], f16)
        ident_s = const.tile([P, P], f32)
        nc.sync.dma_start(out=wl_s[:], in_=w_l[:])
        nc.sync.dma_start(out=wr_s[:], in_=w_r[:])
        nc.sync.dma_start(out=att_s[:], in_=att_d[:])
        nc.sync.dma_start(out=iota_s[:], in_=iota_d[:])
        nc.sync.dma_start(out=iota8_s[:], in_=iota8_d[:])
        nc.sync.dma_start(out=ident_s[:], in_=ident_d[:])
        aff_s = const.tile([P, 3 * OUT], f32)
        nc.sync.dma_start(out=aff_s[:], in_=aff_d[:])
        eps_s = const.tile([P, 1], f32)
        nc.sync.dma_start(out=eps_s[:], in_=eps_d[:])

        # PE warmup: observe DMA-loaded constants once on the PE clock
        with tc.tile_pool(name="warm", bufs=1, space="PSUM") as warm:
            warm_p = warm.tile([P, P], f32)
            nc.tensor.transpose(out=warm_p[:], in_=ident_s[:],
                                identity=ident_s[:])
            nc.tensor.matmul(out=warm_p[:, :OUT], lhsT=ident_s[:],
                             rhs=wl_s[:], start=True, stop=True)
            nc.tensor.matmul(out=warm_p[:, :OUT], lhsT=ident_s[:],
                             rhs=wr_s[:], start=True, stop=True)

        # ------------------------------------------------------------------
        # Phase 1: projection tables (fp16)
        # ------------------------------------------------------------------
        def project(src_ap, n_rows, outs):
            # outs: list of (w_tile, dst_dram)
            with tc.tile_pool(name="p1", bufs=3) as p1, \
                 tc.tile_pool(name="p1ps", bufs=2, space="PSUM") as p1ps:
                n_blk = (n_rows + 511) // 512
                for b in range(n_blk):
                    r0 = b * 512
                    rows = min(512, n_rows - r0)
                    n_j = (rows + P - 1) // P
                    xin = p1.tile([P, 4, IN], f32, tag="xin")
                    if rows == 512:
                        nc.sync.dma_start(
                            out=xin[:],
                            in_=src_ap[r0 : r0 + 512, :].rearrange(
                                "(j p) f -> p j f", p=P),
                        )
                    else:
                        for j in range(n_j):
                            jr = min(P, rows - j * P)
                            nc.sync.dma_start(
                                out=xin[:jr, j, :],
                                in_=src_ap[r0 + j * P : r0 + j * P + jr, :])
                    stgs = [p1.tile([P, 4, OUT], f16, tag=f"stg{k}",
                                    name=f"stg{k}_{b}")
                            for k in range(len(outs))]
                    o_ps = [p1ps.tile([P, 4, OUT], f32, tag=f"op{k}",
                                      name=f"op{k}_{b}")
                            for k in range(len(outs))]
                    for j in range(n_j):
                        jr = min(P, rows - j * P)
                        xt_p = p1ps.tile([P, P], f32, tag="xt")
                        nc.tensor.transpose(
                            out=xt_p[:, :jr], in_=xin[:jr, j, :],
                            identity=ident_s[:jr, :jr])
                        xt_s = p1.tile([P, P], f32, tag="xts")
                        nc.scalar.copy(out=xt_s[:, :jr], in_=xt_p[:, :jr])
                        for k, (w_tile, _) in enumerate(outs):
                            # one accumulation group per PSUM bank: start
                            # zeroes the whole 2KB zero-region, so only j==0
                            # may set it
                            nc.tensor.matmul(
                                out=o_ps[k][:jr, j, :], lhsT=xt_s[:, :jr],
                                rhs=w_tile[:], start=(j == 0),
                                stop=(j == n_j - 1))
                    for k in range(len(outs)):
                        nc.vector.tensor_copy(out=stgs[k][:, :n_j, :],
                                              in_=o_ps[k][:, :n_j, :])
                    for k, (_, dst) in enumerate(outs):
                        if rows == 512:
                            nc.sync.dma_start(
                                out=dst[r0 : r0 + 512, :].rearrange(
                                    "(j p) f -> p j f", p=P),
                                in_=stgs[k][:])
                        else:
                            for j in range(n_j):
                                jr = min(P, rows - j * P)
                                nc.sync.dma_start(
                                    out=dst[r0 + j * P : r0 + j * P + jr, :],
                                    in_=stgs[k][:jr, j, :])

        project(x_full[:], n_nodes, [(wl_s, xl_tab[:])])
        project(x_own[:], own_pad, [(wl_s, xlo_tab[:]), (wr_s, xr_tab[:])])

        # ------------------------------------------------------------------
        # Phase 2: chunked edge processing
        # ------------------------------------------------------------------
        with tc.tile_pool(name="p2", bufs=3) as p2, \
             tc.tile_pool(name="stg2", bufs=2) as stg2, \
             tc.tile_pool(name="gps", bufs=2, space="PSUM") as gps:

            att_b8 = att_s[:][:, None, :].to_broadcast((P, T_SUB, OUT))
            iota_b8 = iota8_s[:].rearrange("p (j d) -> p j d", j=T_SUB)

            state = {"gq": 0}
            sb_off = 0
            for ci, ch in enumerate(chunks):
                gs, nw = ch["gs"], ch["nw"]
                g0 = gs[0]
                n_sub = ch["n_sub"]
                qsub = ch["qsub"]
                w_ch = ch["n_edge"] * 8 + n_sub * 10
                sb_t = stg2.tile([P, sbw_max], i16, tag="sb")
                nc.sync.dma_start(out=sb_t[:, :w_ch],
                                  in_=sb_d[:, sb_off : sb_off + w_ch])
                sb_off += w_ch

                xl_st = stg2.tile([P, nsub_max, OUT], f16, tag="xl")
                xr_st = stg2.tile([P, nsub_max, OUT], f16, tag="xr")
                cap = MAX_GATHER // P
                # gathers rotate across the 4 SWDGE queues so queue-ring
                # reclaim (waiting on the previous gather's DMA) overlaps
                # with the other queues' descriptor generation
                # xl gathers (per quarter section, capped)
                off_w = 0
                si = 0
                for qq in range(N_SPLIT):
                    qs = qsub[qq]
                    src = xl_tab[qq * q_rows : min((qq + 1) * q_rows,
                                                   n_nodes), :]
                    for o in range(0, qs, cap):
                        n = min(cap, qs - o)
                        nc.gpsimd.dma_gather(
                            xl_st[:, si + o : si + o + n, :], src,
                            sb_t[:, off_w + o * 8 : off_w + (o + n) * 8],
                            n * P, n * P, OUT,
                            queue_num=state["gq"] % 4)
                        state["gq"] += 1
                    off_w += qs * 8
                    si += qs
                # xr gathers (capped)
                for o in range(0, n_sub, cap):
                    n = min(cap, n_sub - o)
                    nc.gpsimd.dma_gather(
                        xr_st[:, o : o + n, :], xr_tab[:],
                        sb_t[:, off_w + o * 8 : off_w + (o + n) * 8],
                        n * P, n * P, OUT,
                        queue_num=state["gq"] % 4)
                    state["gq"] += 1
                off_w += n_sub * 8
                dl_rel = sb_t[:, off_w : off_w + n_sub].bitcast(f16)
                off_w += n_sub
                dl_chunk = sb_t[:, off_w : off_w + n_sub].bitcast(f16)

                # self blocks (own-group rows, contiguous)
                xl_sf = p2.tile([P, G_CHUNK, OUT], f16, tag="xlsf")
                xr_sf = p2.tile([P, G_CHUNK, OUT], f16, tag="xrsf")
                nc.sync.dma_start(
                    out=xl_sf[:, :nw, :],
                    in_=xlo_tab[g0 * P : (g0 + nw) * P, :].rearrange(
                        "(j p) f -> p j f", p=P))
                nc.sync.dma_start(
                    out=xr_sf[:, :nw, :],
                    in_=xr_tab[g0 * P : (g0 + nw) * P, :].rearrange(
                        "(j p) f -> p j f", p=P))
                m_s = p2.tile([P, G_CHUNK, OUT], f16, tag="ms")
                nc.vector.tensor_tensor(out=m_s[:, :nw, :],
                                        in0=xl_sf[:, :nw, :],
                                        in1=xr_sf[:, :nw, :], op=Alu.add)
                nc.vector.scalar_tensor_tensor(
                    out=m_s[:, :nw, :], in0=m_s[:, :nw, :], scalar=NEG_SLOPE,
                    in1=m_s[:, :nw, :], op0=Alu.mult, op1=Alu.max)
                u_s = p2.tile([P, G_CHUNK, OUT], f16, tag="us")
                nc.vector.tensor_tensor(
                    out=u_s[:, :nw, :], in0=m_s[:, :nw, :],
                    in1=att_s[:][:, None, :].to_broadcast((P, nw, OUT)),
                    op=Alu.mult)
                e_s = p2.tile([P, G_CHUNK, H], f32, tag="es")
                nc.vector.tensor_reduce(
                    out=e_s[:, :nw, :].rearrange("p j h -> p (j h)"),
                    in_=u_s[:, :nw, :].rearrange("p j (h d) -> p (j h) d", h=H),
                    axis=mybir.AxisListType.X, op=Alu.add)
                ex_s = p2.tile([P, G_CHUNK, H], f16, tag="exs")
                nc.scalar.activation(out=ex_s[:, :nw, :], in_=e_s[:, :nw, :],
                                     func=Act.Exp)
                swt = p2.tile([P, G_CHUNK, OUT], f32, tag="swt")
                nc.vector.tensor_tensor(
                    out=swt[:, :nw, :].rearrange(
                        "p j (h d) -> p j h d", h=H),
                    in0=xl_sf[:, :nw, :].rearrange(
                        "p j (h d) -> p j h d", h=H),
                    in1=ex_s[:, :nw, :, None].to_broadcast((P, nw, H, D)),
                    op=Alu.mult)

                g_all = gps.tile([P, G_CHUNK, OUT], f32, tag="gall",
                                 name=f"gall{ci}")
                g_psum = [g_all[:, k, :] for k in range(nw)]
                g_den = gps.tile([P, G_CHUNK, H], f32, tag="gden",
                                 name=f"gden{ci}")

                # edge big-tiles
                mm = ch["mm"]
                mi = 0
                for bt in range(n_sub // T_SUB):
                    j0 = bt * T_SUB
                    sl = slice(j0, j0 + T_SUB)
                    m = p2.tile([P, T_SUB, OUT], f16, tag="m")
                    nc.vector.tensor_tensor(out=m[:], in0=xl_st[:, sl, :],
                                            in1=xr_st[:, sl, :], op=Alu.add)
                    t_l = p2.tile([P, T_SUB, OUT], f16, tag="tl")
                    nc.vector.scalar_tensor_tensor(
                        out=t_l[:], in0=m[:], scalar=NEG_SLOPE, in1=m[:],
                        op0=Alu.mult, op1=Alu.max)
                    u = p2.tile([P, T_SUB, OUT], f16, tag="u")
                    nc.vector.tensor_tensor(out=u[:], in0=t_l[:], in1=att_b8,
                                            op=Alu.mult)
                    e = p2.tile([P, T_SUB, H], f32, tag="e")
                    nc.vector.tensor_reduce(
                        out=e[:].rearrange("p j h -> p (j h)"),
                        in_=u[:].rearrange("p j (h d) -> p (j h) d", h=H),
                        axis=mybir.AxisListType.X, op=Alu.add)
                    ex = p2.tile([P, T_SUB, H], f16, tag="ex")
                    nc.scalar.activation(out=ex[:], in_=e[:], func=Act.Exp)
                    w_t = p2.tile([P, T_SUB, OUT], f16, tag="wt")
                    nc.vector.tensor_tensor(
                        out=w_t[:].rearrange("p j (h d) -> p j h d", h=H),
                        in0=xl_st[:, sl, :].rearrange(
                            "p j (h d) -> p j h d", h=H),
                        in1=ex[:, :, :, None].to_broadcast((P, T_SUB, H, D)),
                        op=Alu.mult)
                    s4b = p2.tile([P, T_SUB, P], f16, tag="s4b")
                    nc.vector.tensor_tensor(
                        out=s4b[:], in0=iota_b8,
                        in1=dl_rel[:, sl][:, :, None].to_broadcast(
                            (P, T_SUB, P)),
                        op=Alu.is_equal)
                    while mi < len(mm) and mm[mi][0] < j0 + T_SUB:
                        j, slot, grel, bat, st, sp = mm[mi]
                        if bat:
                            lhsT = s4b[:, j - j0, :]
                        else:
                            s4x = p2.tile([P, P], f16, tag="s4x")
                            nc.vector.scalar_tensor_tensor(
                                out=s4x[:],
                                in0=dl_chunk[:, j][:, None].to_broadcast(
                                    (P, P)),
                                scalar=float(-grel * P),
                                in1=iota_s[:], op0=Alu.add, op1=Alu.is_equal)
                            lhsT = s4x[:]
                        # g_all/g_den hold all 4 groups in one PSUM bank
                        # each: a single accumulation group per bank (start
                        # zeroes the whole 2KB zero-region)
                        st = mi == 0
                        sp = mi == len(mm) - 1
                        nc.tensor.matmul(out=g_psum[slot], lhsT=lhsT,
                                         rhs=w_t[:, j - j0, :],
                                         start=st, stop=sp)
                        # denominator rides the same stationary lhsT
                        nc.tensor.matmul(out=g_den[:, slot, :], lhsT=lhsT,
                                         rhs=ex[:, j - j0, :],
                                         start=st, stop=sp)
                        mi += 1
                assert mi == len(mm)

                # ---- epilogue for this chunk's groups
                stage = p2.tile([P, G_CHUNK, OUT], f32, tag="stage")
                nc.vector.tensor_tensor(out=stage[:, :nw, :],
                                        in0=g_all[:, :nw, :],
                                        in1=swt[:, :nw, :], op=Alu.add)
                rd = p2.tile([P, G_CHUNK, H], f32, tag="rd")
                # flat (j h) views: inner-8 strided APs run ~1 elem/cycle
                nc.vector.tensor_tensor(
                    out=rd[:].rearrange("p j h -> p (j h)"),
                    in0=g_den[:].rearrange("p j h -> p (j h)"),
                    in1=ex_s[:].rearrange("p j h -> p (j h)"), op=Alu.add)
                nc.vector.tensor_scalar_add(
                    rd[:].rearrange("p j h -> p (j h)"),
                    rd[:].rearrange("p j h -> p (j h)"), DEN_EPS)
                nc.vector.reciprocal(
                    rd[:].rearrange("p j h -> p (j h)"),
                    rd[:].rearrange("p j h -> p (j h)"))
                o1 = p2.tile([P, G_CHUNK, OUT], f32, tag="o1")
                nc.vector.tensor_tensor(
                    out=o1[:, :nw, :].rearrange("p j (h d) -> p j h d", h=H),
                    in0=stage[:, :nw, :].rearrange("p j (h d) -> p j h d",
                                                   h=H),
                    in1=rd[:, :nw, :, None].to_broadcast((P, nw, H, D)),
                    op=Alu.mult)
                if use_bias:
                    nc.vector.tensor_tensor(
                        out=o1[:, :nw, :], in0=o1[:, :nw, :],
                        in1=aff_s[:][:, None, 2 * OUT : 3 * OUT].to_broadcast(
                            (P, nw, OUT)),
                        op=Alu.add)
                xres = p2.tile([P, G_CHUNK, OUT], f32, tag="xres")
                nc.sync.dma_start(
                    out=xres[:, :nw, :],
                    in_=x_own[g0 * P : (g0 + nw) * P, :].rearrange(
                        "(j p) f -> p j f", p=P))
                # ELU(o1) + x  (the ELU -1 is dropped: LN is shift-invariant)
                vmin = p2.tile([P, G_CHUNK, OUT], f32, tag="vmin")
                nc.vector.tensor_scalar_min(vmin[:, :nw, :], o1[:, :nw, :],
                                            0.0)
                ev = p2.tile([P, G_CHUNK, OUT], f32, tag="ev")
                nc.scalar.activation(out=ev[:, :nw, :], in_=vmin[:, :nw, :],
                                     func=Act.Exp)
                v = p2.tile([P, G_CHUNK, OUT], f32, tag="v")
                nc.vector.scalar_tensor_tensor(
                    out=v[:, :nw, :], in0=o1[:, :nw, :], scalar=0.0,
                    in1=xres[:, :nw, :], op0=Alu.max, op1=Alu.add)
                nc.vector.tensor_tensor(out=v[:, :nw, :], in0=v[:, :nw, :],
                                        in1=ev[:, :nw, :], op=Alu.add)
                mu = p2.tile([P, G_CHUNK], f32, tag="mu")
                nc.vector.tensor_reduce(out=mu[:, :nw], in_=v[:, :nw, :],
                                        axis=mybir.AxisListType.X, op=Alu.add)
                nc.scalar.mul(out=mu[:, :nw], in_=mu[:, :nw], mul=1.0 / OUT)
                cen = p2.tile([P, G_CHUNK, OUT], f32, tag="cen")
                nc.vector.tensor_tensor(
                    out=cen[:, :nw, :], in0=v[:, :nw, :],
                    in1=mu[:, :nw, None].to_broadcast((P, nw, OUT)),
                    op=Alu.subtract)
                sq = p2.tile([P, G_CHUNK, OUT], f32, tag="sq")
                nc.vector.tensor_tensor(out=sq[:, :nw, :], in0=cen[:, :nw, :],
                                        in1=cen[:, :nw, :], op=Alu.mult)
                var = p2.tile([P, G_CHUNK], f32, tag="var")
                nc.vector.tensor_reduce(out=var[:, :nw], in_=sq[:, :nw, :],
                                        axis=mybir.AxisListType.X, op=Alu.add)
                # std = sqrt(var/OUT + eps) in one ACT op
                nc.scalar.activation(out=var[:, :nw], in_=var[:, :nw],
                                     func=Act.Sqrt, scale=1.0 / OUT,
                                     bias=eps_s[:, 0:1])
                nc.vector.reciprocal(var[:, :nw], var[:, :nw])
                o2 = p2.tile([P, G_CHUNK, OUT], f32, tag="o2")
                nc.vector.tensor_tensor(
                    out=o2[:, :nw, :], in0=cen[:, :nw, :],
                    in1=var[:, :nw, None].to_broadcast((P, nw, OUT)),
                    op=Alu.mult)
                if use_gamma:
                    nc.vector.tensor_tensor(
                        out=o2[:, :nw, :], in0=o2[:, :nw, :],
                        in1=aff_s[:][:, None, 0:OUT].to_broadcast((P, nw, OUT)),
                        op=Alu.mult)
                if use_beta:
                    nc.vector.tensor_tensor(
                        out=o2[:, :nw, :], in0=o2[:, :nw, :],
                        in1=aff_s[:][:, None, OUT : 2 * OUT].to_broadcast(
                            (P, nw, OUT)),
                        op=Alu.add)
                nc.sync.dma_start(
                    out=out_own[g0 * P : (g0 + nw) * P, :].rearrange(
                        "(j p) f -> p j f", p=P),
                    in_=o2[:, :nw, :])

    nc.finalize()
    return nc


# ---------------------------------------------------------------------------
# Host entry point
# ---------------------------------------------------------------------------

TRACE = False
LAST = {}


def kernel(x, edge_index, W_l, b_l, W_r, b_r, att, bias, gamma, beta):
    from concourse.bass_utils import run_bass_kernel_spmd

    x = np.asarray(x, dtype=np.float32)
    n_nodes = x.shape[0]

    pp = _preprocess(np.asarray(edge_index), n_nodes)
    per, own_pad = pp["per"], pp["own_pad"]

    use_bias = bool(np.any(bias))
    use_gamma = bool(np.any(np.asarray(gamma) != 1.0))
    use_beta = bool(np.any(beta))
    # b_l/b_r fold into the tables via host-side? They are zeros in this
    # problem; fall back to adding on host if nonzero.
    assert not np.any(b_l) and not np.any(b_r), "nonzero proj bias unsupported"

    nc = _build_program(n_nodes, pp, use_bias, use_gamma, use_beta)

    att_b = np.tile(np.asarray(att, np.float16).reshape(1, OUT), (P, 1))
    iota_b = np.tile(np.arange(P, dtype=np.float16)[None, :], (P, 1))
    iota8_b = np.tile(np.arange(P, dtype=np.float16)[None, :], (P, T_SUB))
    ident = np.eye(P, dtype=np.float32)
    aff = np.zeros((P, 3 * OUT), dtype=np.float32)
    aff[:, 0:OUT] = np.asarray(gamma, np.float32)[None, :]
    aff[:, OUT : 2 * OUT] = np.asarray(beta, np.float32)[None, :]
    aff[:, 2 * OUT : 3 * OUT] = np.asarray(bias, np.float32)[None, :]

    in_maps = []
    for c in range(N_CORES):
        x_own = np.zeros((own_pad, IN), dtype=np.float32)
        x_own[:per] = x[c * per : (c + 1) * per]
        in_maps.append({
            "x_full": x,
            "x_own": x_own,
            "w_l": np.asarray(W_l, dtype=np.float32),
            "w_r": np.asarray(W_r, dtype=np.float32),
            "sb": pp["sideband"][c],
            "att_b": att_b,
            "iota_b": iota_b,
            "iota8_b": iota8_b,
            "ident": ident,
            "aff": aff,
            "eps_b": np.full((P, 1), LN_EPS, dtype=np.float32),
        })

    res = run_bass_kernel_spmd(nc, in_maps, list(range(N_CORES)), trace=TRACE)
    LAST["res"] = res
    outs = [res.results[c]["out_own"][:per] for c in range(N_CORES)]
    return np.concatenate(outs, axis=0).astype(np.float32)
